# revision 28
# baseline (speedup 1.0000x reference)
"""Grouped-experts MLP (MoE) kernel for Trainium2, expert-parallel over 8 cores.

Problem: x[B=2, E=8, N=1024, D=1024]; per expert e:
    out[:, e] = GELU(x[:, e] @ w1[e] + b1[e]) @ w2[e] + b2[e]
with w1[e]: [D=1024, H=4096], w2[e]: [H=4096, D=1024].

Sharding: expert axis across the 8 NeuronCores (core e owns expert e).
The host performs the "all-to-all": it hands core e the slab x[:, e]
(pre-transposed to [D, T] and cast to bf16) plus expert e's weights,
and reassembles the full output afterward.

Per-core kernel (T = B*N = 2048 tokens), all matmul operands bf16 with
fp32 PSUM accumulation (bf16 enables the PE's fast-weight-load path, so
LDWEIGHTS hides completely under the 512-col matmul streams; fp32 made
LDWEIGHTS the pole at ~224ns vs the 213ns stream):
  - both weight matrices are SBUF-resident for the whole kernel (loaded
    once: ~30MB total HBM traffic vs 84MB when streaming per half).
  - tokens are processed in 4 quarters of 512. Layer 1 computes
    hT[h][128, 512] per H-tile h (PSUM accum over the 8 D-tiles), with
    GELU + b1 fused into the PSUM->SBUF eviction on the scalar engine,
    writing bf16.
  - layer 2 accumulates all 32 H-tiles of a [128 token, 512 dcol] output
    tile directly in PSUM (32-matmul groups), then adds b2 on the DVE
    while copying to SBUF, and DMAs straight out.
"""

import numpy as np
import ml_dtypes

import concourse.bacc as bacc
import concourse.mybir as mybir
import concourse.tile as tile
from concourse.bass_utils import run_bass_kernel_spmd

B, E, N, D, H = 2, 8, 1024, 1024, 4096
T = B * N          # tokens per expert
P = 128
N_CORES = 8

TQ = 512           # tokens per quarter
NQ = T // TQ       # 4
KD = D // P        # 8 k-tiles over D (layer-1 contraction)
KH = H // P        # 32 k-tiles over H (layer-2 contraction)
HC = 8             # w1 column chunks of 512
HS = 4             # h-subtiles per w1 chunk
DC = D // 512      # 2 output column chunks
NTS = TQ // P      # 4 token subtiles per quarter

F32 = mybir.dt.float32
BF16 = mybir.dt.bfloat16
GELU = mybir.ActivationFunctionType.Gelu
BF16_NP = ml_dtypes.bfloat16


def build_nc():
    nc = bacc.Bacc("TRN2", target_bir_lowering=False, debug=False)

    xT = nc.dram_tensor("xT", [D, T], BF16, kind="ExternalInput")
    w1 = nc.dram_tensor("w1", [D, H], BF16, kind="ExternalInput")
    b1 = nc.dram_tensor("b1", [P, KH], F32, kind="ExternalInput")
    w2 = nc.dram_tensor("w2", [H, D], BF16, kind="ExternalInput")
    b2 = nc.dram_tensor("b2", [P, D], F32, kind="ExternalInput")
    out = nc.dram_tensor("out", [T, D], F32, kind="ExternalOutput")

    with tile.TileContext(nc) as tc:
        with (
            tc.tile_pool(name="const", bufs=1) as constp,
            tc.tile_pool(name="xp", bufs=2) as xp,
            tc.tile_pool(name="w1p", bufs=1) as w1p,
            tc.tile_pool(name="w2p", bufs=1) as w2p,
            tc.tile_pool(name="hTp", bufs=1) as hTp,
            tc.tile_pool(name="stp", bufs=4) as stp,
            tc.tile_pool(name="ps1p", bufs=4, space="PSUM") as ps1p,
            tc.tile_pool(name="ps2p", bufs=3, space="PSUM") as ps2p,
            tc.tile_pool(name="warmps", bufs=1, space="PSUM") as warmps,
        ):
            # HAM pre-warm: dependency-free matmuls on scratch SBUF keep
            # the PE busy through the ~13us DMA startup so the activity
            # monitor un-throttles the clock (1.2 -> 2.4 GHz) before real
            # work arrives. Garbage data, never read.
            warm_src = constp.tile([P, 512], BF16, name="warm_src")
            nc.vector.memset(warm_src[:], 0.0)
            warm_ps = warmps.tile([P, 512], F32, name="warm_ps")

            def warm_mms(n):
                for i in range(n):
                    nc.tensor.matmul(
                        warm_ps[:], warm_src[:, 0:P], warm_src[:],
                        start=(i == 0), stop=(i == n - 1))

            warm_mms(20)

            def load_xq(q):
                tiles = []
                for k in range(KD):
                    t = xp.tile([P, TQ], BF16, name=f"x{q}_{k}", tag=f"x_{k}")
                    nc.sync.dma_start(
                        t[:], xT[k * P:(k + 1) * P, q * TQ:(q + 1) * TQ])
                    tiles.append(t)
                return tiles

            # startup: quarter-0 activations interleaved with w1's first
            # column chunk so the PE can start as soon as they land
            xq_tiles = [None] * NQ
            w1t = {}
            xq0 = []
            for k in range(KD):
                t = xp.tile([P, TQ], BF16, name=f"x0_{k}", tag=f"x_{k}")
                nc.sync.dma_start(t[:], xT[k * P:(k + 1) * P, 0:TQ])
                xq0.append(t)
                tw = w1p.tile([P, 512], BF16, name=f"w1_{k}_0", tag=f"w1_{k}_0")
                nc.sync.dma_start(tw[:], w1[k * P:(k + 1) * P, 0:512])
                w1t[(k, 0)] = tw
            xq_tiles[0] = xq0

            # b1 is tiny and first needed ~15us in
            b1sb = constp.tile([P, KH], F32, name="b1sb")
            nc.sync.dma_start(b1sb[:], b1[:])

            # w1 chunk 1 stays [P,512] for startup granularity
            for k in range(KD):
                tw = w1p.tile([P, 512], BF16, name=f"w1_{k}_1", tag=f"w1_{k}_1")
                nc.sync.dma_start(tw[:], w1[k * P:(k + 1) * P, 512:1024])
                w1t[(k, 1)] = tw
            # rest of w1 (resident) as [P,1024] tiles: 2KB DMA rows move
            # ~9% faster than 1KB, which the startup window is bound by
            w1big = {}
            for j in range(3):
                c0 = 1024 + j * 1024
                for k in range(KD):
                    tw = w1p.tile([P, 1024], BF16,
                                  name=f"w1b_{k}_{j}", tag=f"w1b_{k}_{j}")
                    nc.sync.dma_start(
                        tw[:], w1[k * P:(k + 1) * P, c0:c0 + 1024])
                    w1big[(k, j)] = tw

            def w1_ap(k, h):
                col = h * P
                if col < 1024:
                    off = col % 512
                    return w1t[(k, col // 512)][:, off:off + P]
                j, off = (col - 1024) // 1024, (col - 1024) % 1024
                return w1big[(k, j)][:, off:off + P]

            # w2 (resident) as [P,1024] tiles (2KB rows); layer 2 runs
            # dc-major so only the dc=0 slices are needed when it starts
            # (~83us in), by which point the whole stream has landed
            w2t = []
            b2sb = None
            for k in range(KH):
                tw = w2p.tile([P, D], BF16, name=f"w2_{k}", tag=f"w2_{k}")
                nc.sync.dma_start(tw[:], w2[k * P:(k + 1) * P, :])
                w2t.append(tw)
                if k == 15:
                    b2sb = constp.tile([P, D], F32, name="b2sb")
                    nc.sync.dma_start(b2sb[:], b2[:])

            for q in range(NQ):
                xq = xq_tiles[q]

                # layer 1: hT[h] = GELU(w1[:, h-tile].T @ xq + b1[h-tile])
                hTt = []
                for h in range(KH):
                    hc, hs = h // HS, h % HS
                    ps = ps1p.tile([P, TQ], F32, name="ps1", tag="ps1")
                    for k in range(KD):
                        nc.tensor.matmul(
                            ps[:],
                            w1_ap(k, h),
                            xq[k][:],
                            start=(k == 0),
                            stop=(k == KD - 1),
                        )
                    ht = hTp.tile([P, TQ], BF16, name=f"hT_{h}", tag=f"hT_{h}")
                    nc.scalar.activation(
                        ht[:], ps[:], GELU, bias=b1sb[:, h:h + 1])
                    hTt.append(ht)
                    # one PE-filler between early groups: absorbs the DMA
                    # ramp so HAM never sees a long idle window
                    if q == 0 and 2 < h < 12:
                        warm_mms(1)
                    # prefetch next quarter's activations mid-layer-1, when
                    # the previous quarter's reads are long done
                    if h == 8 and q + 1 < NQ:
                        xq_tiles[q + 1] = load_xq(q + 1)

                # layer 2: out tile [128 tok, 512 dcol] accumulates all 32
                # H-tiles in PSUM, then +b2 on the DVE and straight to DRAM
                for dc in range(DC):
                    sl = slice(dc * 512, (dc + 1) * 512)
                    for ts in range(NTS):
                        t0 = q * TQ + ts * P
                        ps = ps2p.tile([P, 512], F32, name="ps2", tag="ps2")
                        for k in range(KH):
                            nc.tensor.matmul(
                                ps[:],
                                hTt[k][:, ts * P:(ts + 1) * P],
                                w2t[k][:, sl],
                                start=(k == 0),
                                stop=(k == KH - 1),
                            )
                        st = stp.tile([P, 512], F32, name="stage", tag="stage")
                        nc.vector.tensor_add(st[:], b2sb[:, sl], ps[:])
                        nc.sync.dma_start(out[t0:t0 + P, sl], st[:])

    nc.compile()
    return nc


def make_in_map(x_e, w1_e, b1_e, w2_e, b2_e):
    """Per-core input map from one expert's full-precision slabs."""
    xT = np.ascontiguousarray(x_e.reshape(T, D).T).astype(BF16_NP)
    return {
        "xT": xT,
        "w1": np.ascontiguousarray(w1_e).astype(BF16_NP),
        "b1": np.ascontiguousarray(b1_e.reshape(KH, P).T),
        "w2": np.ascontiguousarray(w2_e).astype(BF16_NP),
        "b2": np.ascontiguousarray(
            np.broadcast_to(b2_e.reshape(1, D), (P, D))),
    }


_NC_CACHE = None


def _get_nc():
    global _NC_CACHE
    if _NC_CACHE is None:
        _NC_CACHE = build_nc()
    return _NC_CACHE


def kernel(x, w1, b1, w2, b2, trace=False):
    x = np.asarray(x, dtype=np.float32)
    w1 = np.asarray(w1, dtype=np.float32)
    b1 = np.asarray(b1, dtype=np.float32)
    w2 = np.asarray(w2, dtype=np.float32)
    b2 = np.asarray(b2, dtype=np.float32)

    nc = _get_nc()
    in_maps = [
        make_in_map(x[:, e], w1[e], b1[e], w2[e], b2[e]) for e in range(N_CORES)
    ]
    res = run_bass_kernel_spmd(
        nc, in_maps, core_ids=list(range(N_CORES)), trace=trace)
    out = np.empty((B, E, N, D), np.float32)
    for e in range(N_CORES):
        out[:, e] = res.results[e]["out"].reshape(B, N, D)
    if trace:
        return out, res
    return out


# revision 29
# speedup vs baseline: 1.1991x; 1.1991x over previous
"""Grouped-experts MLP (MoE) kernel for Trainium2, expert-parallel over 8 cores.

Problem: x[B=2, E=8, N=1024, D=1024]; per expert e:
    out[:, e] = GELU(x[:, e] @ w1[e] + b1[e]) @ w2[e] + b2[e]
with w1[e]: [D=1024, H=4096], w2[e]: [H=4096, D=1024].

Sharding: expert axis across the 8 NeuronCores (core e owns expert e).
The host performs the "all-to-all": it hands core e the slab x[:, e]
(pre-transposed to [D, T] and cast to bf16) plus expert e's weights,
and reassembles the full output afterward.

Per-core kernel (T = B*N = 2048 tokens), all matmul operands bf16 with
fp32 PSUM accumulation (bf16 enables the PE's fast-weight-load path, so
LDWEIGHTS hides completely under the 512-col matmul streams; fp32 made
LDWEIGHTS the pole at ~224ns vs the 213ns stream):
  - both weight matrices are SBUF-resident for the whole kernel (loaded
    once: ~30MB total HBM traffic vs 84MB when streaming per half).
  - tokens are processed in 4 quarters of 512. Layer 1 computes
    hT[h][128, 512] per H-tile h (PSUM accum over the 8 D-tiles), with
    GELU + b1 fused into the PSUM->SBUF eviction on the scalar engine,
    writing bf16.
  - layer 2 accumulates all 32 H-tiles of a [128 token, 512 dcol] output
    tile directly in PSUM (32-matmul groups), then adds b2 on the DVE
    while copying to SBUF, and DMAs straight out.
"""

import numpy as np
import ml_dtypes

import concourse.bacc as bacc
import concourse.mybir as mybir
import concourse.tile as tile
from concourse.bass_utils import run_bass_kernel_spmd

B, E, N, D, H = 2, 8, 1024, 1024, 4096
T = B * N          # tokens per expert
P = 128
N_CORES = 8

TQ = 512           # tokens per quarter
NQ = T // TQ       # 4
KD = D // P        # 8 k-tiles over D (layer-1 contraction)
KH = H // P        # 32 k-tiles over H (layer-2 contraction)
HC = 8             # w1 column chunks of 512
HS = 4             # h-subtiles per w1 chunk
DC = D // 512      # 2 output column chunks
NTS = TQ // P      # 4 token subtiles per quarter

F32 = mybir.dt.float32
BF16 = mybir.dt.bfloat16
GELU = mybir.ActivationFunctionType.Gelu
BF16_NP = ml_dtypes.bfloat16


def build_nc():
    nc = bacc.Bacc("TRN2", target_bir_lowering=False, debug=False)

    xT = nc.dram_tensor("xT", [D, T], BF16, kind="ExternalInput")
    w1 = nc.dram_tensor("w1", [D, H], BF16, kind="ExternalInput")
    b1 = nc.dram_tensor("b1", [P, KH], F32, kind="ExternalInput")
    w2 = nc.dram_tensor("w2", [H, D], BF16, kind="ExternalInput")
    b2 = nc.dram_tensor("b2", [P, D], F32, kind="ExternalInput")
    out = nc.dram_tensor("out", [T, D], F32, kind="ExternalOutput")

    with tile.TileContext(nc) as tc:
        with (
            tc.tile_pool(name="const", bufs=1) as constp,
            tc.tile_pool(name="xp", bufs=2) as xp,
            tc.tile_pool(name="w1p", bufs=1) as w1p,
            tc.tile_pool(name="w2p", bufs=1) as w2p,
            tc.tile_pool(name="hTp", bufs=1) as hTp,
            tc.tile_pool(name="stp", bufs=4) as stp,
            tc.tile_pool(name="ps1p", bufs=4, space="PSUM") as ps1p,
            tc.tile_pool(name="ps2p", bufs=3, space="PSUM") as ps2p,
            tc.tile_pool(name="warmps", bufs=1, space="PSUM") as warmps,
        ):
            # HAM pre-warm: ~48 dependency-free matmuls on scratch SBUF keep
            # the PE busy through the ~13us DMA startup so the activity
            # monitor un-throttles the clock (1.2 -> 2.4 GHz) before real
            # work arrives. Garbage data, never read.
            warm_src = constp.tile([P, 512], BF16, name="warm_src")
            nc.vector.memset(warm_src[:], 0.0)
            warm_ps = warmps.tile([P, 512], F32, name="warm_ps")

            def warm_mms(n):
                for i in range(n):
                    nc.tensor.matmul(
                        warm_ps[:], warm_src[:, 0:P], warm_src[:],
                        start=(i == 0), stop=(i == n - 1))

            warm_mms(20)

            def load_xq(q):
                tiles = []
                for k in range(KD):
                    t = xp.tile([P, TQ], BF16, name=f"x{q}_{k}", tag=f"x_{k}")
                    nc.sync.dma_start(
                        t[:], xT[k * P:(k + 1) * P, q * TQ:(q + 1) * TQ])
                    tiles.append(t)
                return tiles

            # startup: quarter-0 activations interleaved with w1's first
            # column chunk so the PE can start as soon as they land
            xq_tiles = [None] * NQ
            w1t = {}
            xq0 = []
            for k in range(KD):
                t = xp.tile([P, TQ], BF16, name=f"x0_{k}", tag=f"x_{k}")
                nc.sync.dma_start(t[:], xT[k * P:(k + 1) * P, 0:TQ])
                xq0.append(t)
                tw = w1p.tile([P, 512], BF16, name=f"w1_{k}_0", tag=f"w1_{k}_0")
                nc.sync.dma_start(tw[:], w1[k * P:(k + 1) * P, 0:512])
                w1t[(k, 0)] = tw
            xq_tiles[0] = xq0

            # b1 is tiny and first needed ~15us in
            b1sb = constp.tile([P, KH], F32, name="b1sb")
            nc.sync.dma_start(b1sb[:], b1[:])

            # rest of w1 (resident)
            for hc in range(1, HC):
                for k in range(KD):
                    tw = w1p.tile([P, 512], BF16,
                                  name=f"w1_{k}_{hc}", tag=f"w1_{k}_{hc}")
                    nc.sync.dma_start(
                        tw[:], w1[k * P:(k + 1) * P, hc * 512:(hc + 1) * 512])
                    w1t[(k, hc)] = tw

            # w2 (resident), streamed dc-half-major: layer 2 runs dc-major,
            # so its first groups only need the dc=0 half, which arrives
            # well before it is consumed (~83us in)
            w2t = {}
            b2sb = None
            for dc in range(DC):
                for k in range(KH):
                    tw = w2p.tile([P, 512], BF16,
                                  name=f"w2_{k}_{dc}", tag=f"w2_{k}_{dc}")
                    nc.sync.dma_start(
                        tw[:],
                        w2[k * P:(k + 1) * P, dc * 512:(dc + 1) * 512])
                    w2t[(k, dc)] = tw
                if dc == 0:
                    b2sb = constp.tile([P, D], F32, name="b2sb")
                    nc.sync.dma_start(b2sb[:], b2[:])

            for q in range(NQ):
                xq = xq_tiles[q]

                # layer 1: hT[h] = GELU(w1[:, h-tile].T @ xq + b1[h-tile])
                hTt = []
                for h in range(KH):
                    hc, hs = h // HS, h % HS
                    ps = ps1p.tile([P, TQ], F32, name="ps1", tag="ps1")
                    for k in range(KD):
                        nc.tensor.matmul(
                            ps[:],
                            w1t[(k, hc)][:, hs * P:(hs + 1) * P],
                            xq[k][:],
                            start=(k == 0),
                            stop=(k == KD - 1),
                        )
                    ht = hTp.tile([P, TQ], BF16, name=f"hT_{h}", tag=f"hT_{h}")
                    nc.scalar.activation(
                        ht[:], ps[:], GELU, bias=b1sb[:, h:h + 1])
                    hTt.append(ht)
                    # one PE-filler between early groups: absorbs the DMA
                    # ramp so HAM never sees a long idle window
                    if q == 0 and 2 < h < 12:
                        warm_mms(1)
                    # prefetch next quarter's activations mid-layer-1, when
                    # the previous quarter's reads are long done
                    if h == 8 and q + 1 < NQ:
                        xq_tiles[q + 1] = load_xq(q + 1)

                # layer 2: out tile [128 tok, 512 dcol] accumulates all 32
                # H-tiles in PSUM, then +b2 on the DVE and straight to DRAM
                for dc in range(DC):
                    sl = slice(dc * 512, (dc + 1) * 512)
                    for ts in range(NTS):
                        t0 = q * TQ + ts * P
                        ps = ps2p.tile([P, 512], F32, name="ps2", tag="ps2")
                        for k in range(KH):
                            nc.tensor.matmul(
                                ps[:],
                                hTt[k][:, ts * P:(ts + 1) * P],
                                w2t[(k, dc)][:],
                                start=(k == 0),
                                stop=(k == KH - 1),
                            )
                        st = stp.tile([P, 512], F32, name="stage", tag="stage")
                        nc.vector.tensor_add(st[:], b2sb[:, sl], ps[:])
                        nc.sync.dma_start(out[t0:t0 + P, sl], st[:])

    nc.compile()
    return nc


def make_in_map(x_e, w1_e, b1_e, w2_e, b2_e):
    """Per-core input map from one expert's full-precision slabs."""
    xT = np.ascontiguousarray(x_e.reshape(T, D).T).astype(BF16_NP)
    return {
        "xT": xT,
        "w1": np.ascontiguousarray(w1_e).astype(BF16_NP),
        "b1": np.ascontiguousarray(b1_e.reshape(KH, P).T),
        "w2": np.ascontiguousarray(w2_e).astype(BF16_NP),
        "b2": np.ascontiguousarray(
            np.broadcast_to(b2_e.reshape(1, D), (P, D))),
    }


_NC_CACHE = None


def _get_nc():
    global _NC_CACHE
    if _NC_CACHE is None:
        _NC_CACHE = build_nc()
    return _NC_CACHE


def kernel(x, w1, b1, w2, b2, trace=False):
    x = np.asarray(x, dtype=np.float32)
    w1 = np.asarray(w1, dtype=np.float32)
    b1 = np.asarray(b1, dtype=np.float32)
    w2 = np.asarray(w2, dtype=np.float32)
    b2 = np.asarray(b2, dtype=np.float32)

    nc = _get_nc()
    in_maps = [
        make_in_map(x[:, e], w1[e], b1[e], w2[e], b2[e]) for e in range(N_CORES)
    ]
    res = run_bass_kernel_spmd(
        nc, in_maps, core_ids=list(range(N_CORES)), trace=trace)
    out = np.empty((B, E, N, D), np.float32)
    for e in range(N_CORES):
        out[:, e] = res.results[e]["out"].reshape(B, N, D)
    if trace:
        return out, res
    return out
